# revision 12
# baseline (speedup 1.0000x reference)
"""Trainium2 Bass kernel for the Tacotron-style decoder step (nn_Decoder).

Strategy: data-parallel over batch. 128 rows -> 16 rows on each of 8
NeuronCores. Weights are replicated; all layout work (transposes,
padding, bias merging, mel r-slicing) happens on the host in numpy so
every device DMA is a natural, contiguous-stride load.

Per-core device program:
  1. PreNet + GRU + q projection, batched over the 16 rows in
     transposed [*, 16] layout (row index on the matmul free axis).
  2. Bahdanau attention, one row at a time, streaming the two 1MB
     encoder slices per row:
       esp^T [D,T] (host pre-transposed)  -> ScalarE tanh(x + q_d) with
       per-partition bias, u = v'tanh via PE (tanh chunks as lhsT),
       exp on ScalarE with accum_out giving partial softmax sums,
       context = exp_u @ enc via PSUM-accumulated matvecs.
     Softmax skips max-subtraction: |u| <= sum|v_d| ~ 10, safe in fp32.
  3. rnn_in + LSTM1 + LSTM2 + mel projection, batched over 16 rows.
     mel_W is pre-sliced by r on the host (only 80*r of 1600 rows).
"""

import numpy as np

import concourse.bacc as bacc
import concourse.bass as bass
import concourse.tile as tile
from concourse import mybir
from concourse.bass_utils import run_bass_kernel_spmd
from concourse.masks import make_identity

DT = mybir.dt.float32
B, T, D, L, NM, MAXR = 128, 1024, 256, 512, 80, 20
NCORES = 8
BL = B // NCORES  # 16 rows per core
TJ = T // 128     # 8 t-tiles
DK = D // 128     # 2 d-tiles
LK = L // 128     # 4 l-tiles


def _build(r: int):
    nc = bacc.Bacc(None, target_bir_lowering=False)

    def inp(name, shape):
        return nc.declare_dram_parameter(name, list(shape), DT, isOutput=False)

    def outp(name, shape):
        return nc.declare_dram_parameter(name, list(shape), DT, isOutput=True)

    # big streamed inputs
    d_espT = inp("espT", [BL, D, T])       # encoder_seq_proj, per-row transposed
    d_enc = inp("enc", [BL, T, D])         # encoder_seq, natural
    # small per-row state, transposed to [dim, BL]
    d_prenetT = inp("prenetT", [128, BL])  # padded 80 -> 128
    d_attnhT = inp("attnhT", [D, BL])
    d_ctxvT = inp("ctxvT", [D, BL])
    d_r1hT = inp("r1hT", [L, BL])
    d_r2hT = inp("r2hT", [L, BL])
    d_r1cT = inp("r1cT", [L, BL])
    d_r2cT = inp("r2cT", [L, BL])
    # weights, host pre-transposed to [in, out]
    d_wfc1T = inp("wfc1T", [128, 256])     # padded 80 -> 128 on in-dim
    d_wfc2T = inp("wfc2T", [256, 128])
    d_wattnT = inp("wattnT", [D, D])
    d_wgihT = inp("wgihT", [D + D // 2, 3 * D])
    d_wghhT = inp("wghhT", [D, 3 * D])
    d_wrninT = inp("wrninT", [2 * D, L])
    d_wl1ihT = inp("wl1ihT", [L, 4 * L])
    d_wl1hhT = inp("wl1hhT", [L, 4 * L])
    d_wl2ihT = inp("wl2ihT", [L, 4 * L])
    d_wl2hhT = inp("wl2hhT", [L, 4 * L])
    NMEL = NM * r
    d_wmelT = inp("wmelT", [L, NMEL])
    d_vcol = inp("vcol", [D])
    # biases
    d_bfc1 = inp("bfc1", [256])
    d_bfc2 = inp("bfc2", [128])
    d_brz = inp("brz", [2 * D])            # (gru_bih+gru_bhh)[0:2D]
    d_bgin = inp("bgin", [D])              # gru_bih[2D:3D]
    d_bghn = inp("bghn", [D])              # gru_bhh[2D:3D]
    d_brnin = inp("brnin", [L])
    d_bl1 = inp("bl1", [4 * L])            # l1_bih + l1_bhh
    d_bl2 = inp("bl2", [4 * L])
    # outputs
    o_mels = outp("o_mels", [BL, NMEL])
    o_scores = outp("o_scores", [BL, T])
    o_attnh = outp("o_attnh", [BL, D])
    o_r1h = outp("o_r1h", [BL, L])
    o_r2h = outp("o_r2h", [BL, L])
    o_r1c = outp("o_r1c", [BL, L])
    o_r2c = outp("o_r2c", [BL, L])
    o_ctx = outp("o_ctx", [BL, D])
    # internal scratch for softmax-recip broadcast and per-row context rows
    # (engine APs must start at partition 0, so ctx rows go via DRAM)
    d_scr = nc.dram_tensor("scr_recip", [BL], DT)
    d_ctxscr = nc.dram_tensor("scr_ctx", [BL, D], DT)

    AF = mybir.ActivationFunctionType

    with tile.TileContext(nc) as tc:
        with tc.tile_pool(name="consts", bufs=1) as consts, \
             tc.tile_pool(name="states", bufs=1) as states, \
             tc.tile_pool(name="lstmw", bufs=2) as lstmw, \
             tc.tile_pool(name="espp", bufs=2) as espp, \
             tc.tile_pool(name="encp", bufs=2) as encp, \
             tc.tile_pool(name="tanhp", bufs=2) as tanhp, \
             tc.tile_pool(name="attsm", bufs=1) as attsm, \
             tc.tile_pool(name="work", bufs=1) as work:

            # ---------------- constants / small weights ----------------
            ident = consts.tile([128, 128], DT, tag="ident")
            make_identity(nc, ident)
            ones_col = consts.tile([128, 1], DT, tag="ones")
            nc.vector.memset(ones_col, 1.0)

            wfc1 = consts.tile([128, 256], DT, tag="wfc1")
            nc.gpsimd.dma_start(out=wfc1, in_=d_wfc1T[:])
            wfc2 = consts.tile([128, 2, 128], DT, tag="wfc2")
            nc.gpsimd.dma_start(
                out=wfc2, in_=d_wfc2T.rearrange("(k p) n -> p k n", p=128))
            wattn = consts.tile([128, DK, D], DT, tag="wattn")
            nc.gpsimd.dma_start(
                out=wattn, in_=d_wattnT.rearrange("(k p) n -> p k n", p=128))
            wgih = consts.tile([128, 3, 3 * D], DT, tag="wgih")
            nc.gpsimd.dma_start(
                out=wgih, in_=d_wgihT.rearrange("(k p) n -> p k n", p=128))
            wghh = consts.tile([128, DK, 3 * D], DT, tag="wghh")
            nc.gpsimd.dma_start(
                out=wghh, in_=d_wghhT.rearrange("(k p) n -> p k n", p=128))
            wrnin = consts.tile([128, 4, L], DT, tag="wrnin")
            nc.gpsimd.dma_start(
                out=wrnin, in_=d_wrninT.rearrange("(k p) n -> p k n", p=128))
            wmel = consts.tile([128, LK, NMEL], DT, tag="wmel")
            nc.gpsimd.dma_start(
                out=wmel, in_=d_wmelT.rearrange("(k p) n -> p k n", p=128))
            vcol = consts.tile([128, DK], DT, tag="vcol")
            nc.gpsimd.dma_start(
                out=vcol, in_=d_vcol.rearrange("(k p) -> p k", p=128))

            bfc1 = consts.tile([128, 2], DT, tag="bfc1")
            nc.gpsimd.dma_start(
                out=bfc1, in_=d_bfc1.rearrange("(g p) -> p g", p=128))
            bfc2 = consts.tile([128, 1], DT, tag="bfc2")
            nc.gpsimd.dma_start(
                out=bfc2, in_=d_bfc2.rearrange("(g p) -> p g", p=128))
            brz = consts.tile([128, 4], DT, tag="brz")
            nc.gpsimd.dma_start(
                out=brz, in_=d_brz.rearrange("(g p) -> p g", p=128))
            bgin = consts.tile([128, DK], DT, tag="bgin")
            nc.gpsimd.dma_start(
                out=bgin, in_=d_bgin.rearrange("(g p) -> p g", p=128))
            bghn = consts.tile([128, DK], DT, tag="bghn")
            nc.gpsimd.dma_start(
                out=bghn, in_=d_bghn.rearrange("(g p) -> p g", p=128))
            brnin = consts.tile([128, LK], DT, tag="brnin")
            nc.gpsimd.dma_start(
                out=brnin, in_=d_brnin.rearrange("(g p) -> p g", p=128))
            bl1 = consts.tile([128, 16], DT, tag="bl1")
            nc.gpsimd.dma_start(
                out=bl1, in_=d_bl1.rearrange("(g p) -> p g", p=128))
            bl2 = consts.tile([128, 16], DT, tag="bl2")
            nc.gpsimd.dma_start(
                out=bl2, in_=d_bl2.rearrange("(g p) -> p g", p=128))

            prenetT = states.tile([128, BL], DT, tag="prenetT")
            nc.gpsimd.dma_start(out=prenetT, in_=d_prenetT[:])
            attnh_in = states.tile([128, DK, BL], DT, tag="attnh_in")
            nc.gpsimd.dma_start(
                out=attnh_in, in_=d_attnhT.rearrange("(k p) b -> p k b", p=128))
            ctxv = states.tile([128, DK, BL], DT, tag="ctxv")
            nc.gpsimd.dma_start(
                out=ctxv, in_=d_ctxvT.rearrange("(k p) b -> p k b", p=128))
            r1h = states.tile([128, LK, BL], DT, tag="r1h")
            nc.gpsimd.dma_start(
                out=r1h, in_=d_r1hT.rearrange("(k p) b -> p k b", p=128))
            r2h = states.tile([128, LK, BL], DT, tag="r2h")
            nc.gpsimd.dma_start(
                out=r2h, in_=d_r2hT.rearrange("(k p) b -> p k b", p=128))
            r1c = states.tile([128, LK, BL], DT, tag="r1c")
            nc.gpsimd.dma_start(
                out=r1c, in_=d_r1cT.rearrange("(k p) b -> p k b", p=128))
            r2c = states.tile([128, LK, BL], DT, tag="r2c")
            nc.gpsimd.dma_start(
                out=r2c, in_=d_r2cT.rearrange("(k p) b -> p k b", p=128))

            # ---------------- preamble: prenet -> GRU -> q ----------------
            with tc.tile_pool(name="psum_pre", bufs=1, space="PSUM") as pp:
                # PreNet fc1: [256,16] out, in 80(pad 128)
                p1 = work.tile([128, 2, BL], DT, tag="p1")
                ps = pp.tile([128, 2, BL], DT, tag="ps_p1")
                for g in range(2):
                    nc.tensor.matmul(ps[:, g], wfc1[:, bass.ts(g, 128)],
                                     prenetT, start=True, stop=True)
                for g in range(2):
                    nc.scalar.activation(p1[:, g], ps[:, g], AF.Relu,
                                         bias=bfc1[:, g:g + 1])
                # fc2: [128,16]
                p2 = work.tile([128, BL], DT, tag="p2")
                ps2 = pp.tile([128, BL], DT, tag="ps_p2")
                for k in range(2):
                    nc.tensor.matmul(ps2, wfc2[:, k], p1[:, k],
                                     start=(k == 0), stop=(k == 1))
                nc.scalar.activation(p2, ps2, AF.Relu, bias=bfc2[:, 0:1])

                # GRU gi = Wih @ [ctxv; p2]  (in=384: k=0,1 ctxv, k=2 p2)
                ps_gi = pp.tile([128, 6, BL], DT, tag="ps_gi")
                xin = [ctxv[:, 0], ctxv[:, 1], p2]
                for g in range(6):
                    for k in range(3):
                        nc.tensor.matmul(ps_gi[:, g],
                                         wgih[:, k, bass.ts(g, 128)], xin[k],
                                         start=(k == 0), stop=(k == 2))
                # gh = Whh @ attn_hidden
                ps_gh = pp.tile([128, 6, BL], DT, tag="ps_gh")
                for g in range(6):
                    for k in range(DK):
                        nc.tensor.matmul(ps_gh[:, g],
                                         wghh[:, k, bass.ts(g, 128)],
                                         attnh_in[:, k],
                                         start=(k == 0), stop=(k == DK - 1))

                # evacuate gh: tensor_tensor may read at most one PSUM input
                gh_sb = work.tile([128, 6, BL], DT, tag="gh_sb")
                nc.vector.tensor_copy(gh_sb, ps_gh)

                attnh = work.tile([128, DK, BL], DT, tag="attnh")
                for c in range(DK):
                    t_rz = work.tile([128, BL], DT, tag="t_rz")
                    rg = work.tile([128, BL], DT, tag="rg")
                    zg = work.tile([128, BL], DT, tag="zg")
                    ng = work.tile([128, BL], DT, tag="ng")
                    # r gate
                    nc.vector.tensor_add(t_rz, ps_gi[:, c], gh_sb[:, c])
                    nc.scalar.activation(rg, t_rz, AF.Sigmoid,
                                         bias=brz[:, c:c + 1])
                    # z gate
                    nc.vector.tensor_add(t_rz, ps_gi[:, 2 + c], gh_sb[:, 2 + c])
                    nc.scalar.activation(zg, t_rz, AF.Sigmoid,
                                         bias=brz[:, 2 + c:3 + c])
                    # n gate: tanh(gi_n + bgin + rg*(gh_n + bghn))
                    t1 = work.tile([128, BL], DT, tag="t1")
                    nc.vector.tensor_scalar_add(t1, gh_sb[:, 4 + c],
                                                bghn[:, c:c + 1])
                    nc.vector.tensor_mul(t1, rg, t1)
                    nc.vector.tensor_add(t1, ps_gi[:, 4 + c], t1)
                    nc.scalar.activation(ng, t1, AF.Tanh, bias=bgin[:, c:c + 1])
                    # h' = ng + zg*(h - ng)
                    t2 = work.tile([128, BL], DT, tag="t2")
                    nc.vector.tensor_sub(t2, attnh_in[:, c], ng)
                    nc.vector.tensor_mul(t2, zg, t2)
                    nc.vector.tensor_add(attnh[:, c], ng, t2)

                # q = attn_W @ attn_h'
                qT = work.tile([128, DK, BL], DT, tag="qT")
                ps_q = pp.tile([128, DK, BL], DT, tag="ps_q")
                for g in range(DK):
                    for k in range(DK):
                        nc.tensor.matmul(ps_q[:, g],
                                         wattn[:, k, bass.ts(g, 128)],
                                         attnh[:, k],
                                         start=(k == 0), stop=(k == DK - 1))
                for g in range(DK):
                    nc.vector.tensor_copy(qT[:, g], ps_q[:, g])

            # ---------------- attention: stream rows ----------------
            expAll = attsm.tile([128, BL, TJ], DT, tag="expAll")
            spart = attsm.tile([128, BL], DT, tag="spart")
            ctxn = attsm.tile([BL, D], DT, tag="ctxn")

            with tc.tile_pool(name="psum_u", bufs=3, space="PSUM") as pu, \
                 tc.tile_pool(name="psum_c", bufs=3, space="PSUM") as pc:
                for b in range(BL):
                    esp = espp.tile([128, DK, T], DT, tag="esp")
                    nc.sync.dma_start(
                        out=esp,
                        in_=d_espT[b].rearrange("(k p) t -> p k t", p=128))
                    enc = encp.tile([128, TJ, D], DT, tag="enc")
                    nc.sync.dma_start(
                        out=enc,
                        in_=d_enc[b].rearrange("(j p) d -> p j d", p=128))

                    th = tanhp.tile([128, DK, T], DT, tag="th")
                    for k in range(DK):
                        nc.scalar.activation(th[:, k], esp[:, k], AF.Tanh,
                                             bias=qT[:, k, b:b + 1])
                    # u[t] = sum_d v_d * tanh  -> [128t, TJ] psum
                    ps_u = pu.tile([128, TJ], DT, tag="ps_u")
                    for j in range(TJ):
                        for k in range(DK):
                            nc.tensor.matmul(ps_u[:, j:j + 1],
                                             th[:, k, bass.ts(j, 128)],
                                             vcol[:, k:k + 1],
                                             start=(k == 0), stop=(k == DK - 1))
                    # exp + per-partition partial sums
                    nc.scalar.activation(expAll[:, b], ps_u, AF.Exp,
                                         accum_out=spart[:, b:b + 1])
                    # context = exp_u @ enc
                    ps_ctx = pc.tile([1, D], DT, tag="ps_ctx")
                    for j in range(TJ):
                        nc.tensor.matmul(ps_ctx, expAll[:, b, j:j + 1],
                                         enc[:, j],
                                         start=(j == 0), stop=(j == TJ - 1))
                    ctmp = tanhp.tile([1, D], DT, tag="ctmp")
                    nc.scalar.activation(ctmp, ps_ctx, AF.Copy)
                    nc.sync.dma_start(out=d_ctxscr[b:b + 1, :], in_=ctmp)

            # ---------------- softmax normalization ----------------
            nc.sync.dma_start(out=ctxn, in_=d_ctxscr[:])
            with tc.tile_pool(name="psum_n", bufs=1, space="PSUM") as pn:
                ps_s = pn.tile([BL, 1], DT, tag="ps_s")
                nc.tensor.matmul(ps_s, spart, ones_col, start=True, stop=True)
                rcol = attsm.tile([BL, 1], DT, tag="rcol")
                nc.vector.reciprocal(rcol, ps_s)
                # context normalize + out
                nc.vector.tensor_scalar_mul(ctxn, ctxn, rcol)
                nc.sync.dma_start(out=o_ctx[:], in_=ctxn)
                # broadcast 1/S to all 128 partitions via DRAM round-trip
                nc.sync.dma_start(out=d_scr[:], in_=rcol.rearrange("p 1 -> p"))
                rb = attsm.tile([128, BL], DT, tag="rb")
                scr_ap = d_scr[:]
                bcast = bass.AP(tensor=scr_ap.tensor, offset=scr_ap.offset,
                                ap=[[0, 128], [1, BL]])
                nc.sync.dma_start(out=rb, in_=bcast)
                scoresT = attsm.tile([128, BL, TJ], DT, tag="scoresT")
                for b in range(BL):
                    nc.vector.tensor_scalar_mul(scoresT[:, b], expAll[:, b],
                                                rb[:, b:b + 1])
                with nc.allow_non_contiguous_dma("small 64KB scores store"):
                    nc.sync.dma_start(
                        out=o_scores.rearrange("b (j p) -> p b j", p=128),
                        in_=scoresT)

            # transpose context and attn_h to feed batched matmuls
            with tc.tile_pool(name="psum_t1", bufs=2, space="PSUM") as pt:
                ctxT = work.tile([128, DK, BL], DT, tag="ctxT")
                for k in range(DK):
                    ps_t = pt.tile([128, BL], DT, tag="ps_t1")
                    nc.tensor.transpose(ps_t, ctxn[:, bass.ts(k, 128)],
                                        ident[:BL, :BL])
                    nc.vector.tensor_copy(ctxT[:, k], ps_t)
                attnh_nat = work.tile([BL, D], DT, tag="attnh_nat")
                for k in range(DK):
                    ps_t = pt.tile([BL, 128], DT, tag="ps_t2")
                    nc.tensor.transpose(ps_t, attnh[:, k], ident)
                    nc.vector.tensor_copy(attnh_nat[:, bass.ts(k, 128)], ps_t)
                nc.sync.dma_start(out=o_attnh[:], in_=attnh_nat)

            # ---------------- rnn_in + LSTMs + mel ----------------
            with tc.tile_pool(name="psum_l", bufs=1, space="PSUM") as pl, \
                 tc.tile_pool(name="psum_t2", bufs=2, space="PSUM") as pt:
                xT = work.tile([128, LK, BL], DT, tag="xT")
                ps_x = pl.tile([128, LK, BL], DT, tag="ps_x")
                xin2 = [ctxT[:, 0], ctxT[:, 1], attnh[:, 0], attnh[:, 1]]
                for g in range(LK):
                    for k in range(4):
                        nc.tensor.matmul(ps_x[:, g],
                                         wrnin[:, k, bass.ts(g, 128)], xin2[k],
                                         start=(k == 0), stop=(k == 3))
                for g in range(LK):
                    nc.vector.tensor_scalar_add(xT[:, g], ps_x[:, g],
                                                brnin[:, g:g + 1])

                def lstm_cell(tag, d_wih, d_whh, xtiles, htile, ctile, bl,
                              o_h, o_c):
                    wih = lstmw.tile([128, LK, 4 * L], DT, tag="wlstm")
                    nc.gpsimd.dma_start(
                        out=wih, in_=d_wih.rearrange("(k p) n -> p k n", p=128))
                    whh = lstmw.tile([128, LK, 4 * L], DT, tag="wlstm")
                    nc.gpsimd.dma_start(
                        out=whh, in_=d_whh.rearrange("(k p) n -> p k n", p=128))
                    psA = pl.tile([128, 8, BL], DT, tag=f"psA_{tag}")
                    psB = pl.tile([128, 8, BL], DT, tag=f"psB_{tag}")
                    for gg in range(16):
                        ps = psA[:, gg] if gg < 8 else psB[:, gg - 8]
                        for k in range(LK):
                            nc.tensor.matmul(ps, wih[:, k, bass.ts(gg, 128)],
                                             xtiles[k],
                                             start=(k == 0), stop=False)
                        for k in range(LK):
                            nc.tensor.matmul(ps, whh[:, k, bass.ts(gg, 128)],
                                             htile[:, k],
                                             start=False, stop=(k == LK - 1))
                    hT = work.tile([128, LK, BL], DT, tag=f"hT_{tag}")
                    cT = work.tile([128, LK, BL], DT, tag=f"cT_{tag}")
                    h_nat = work.tile([BL, L], DT, tag=f"hnat_{tag}")
                    c_nat = work.tile([BL, L], DT, tag=f"cnat_{tag}")
                    for c in range(LK):
                        si = work.tile([128, BL], DT, tag="si")
                        sf = work.tile([128, BL], DT, tag="sf")
                        tg = work.tile([128, BL], DT, tag="tg")
                        so = work.tile([128, BL], DT, tag="so")
                        nc.scalar.activation(si, psA[:, c], AF.Sigmoid,
                                             bias=bl[:, c:c + 1])
                        nc.scalar.activation(sf, psA[:, 4 + c], AF.Sigmoid,
                                             bias=bl[:, 4 + c:5 + c])
                        nc.scalar.activation(tg, psB[:, c], AF.Tanh,
                                             bias=bl[:, 8 + c:9 + c])
                        nc.scalar.activation(so, psB[:, 4 + c], AF.Sigmoid,
                                             bias=bl[:, 12 + c:13 + c])
                        nc.vector.tensor_mul(sf, sf, ctile[:, c])
                        nc.vector.tensor_mul(si, si, tg)
                        nc.vector.tensor_add(cT[:, c], sf, si)
                        nc.scalar.activation(tg, cT[:, c], AF.Tanh)
                        nc.vector.tensor_mul(hT[:, c], so, tg)
                        ps_t = pt.tile([BL, 128], DT, tag="ps_tr")
                        nc.tensor.transpose(ps_t, hT[:, c], ident)
                        nc.vector.tensor_copy(h_nat[:, bass.ts(c, 128)], ps_t)
                        ps_t = pt.tile([BL, 128], DT, tag="ps_tr")
                        nc.tensor.transpose(ps_t, cT[:, c], ident)
                        nc.vector.tensor_copy(c_nat[:, bass.ts(c, 128)], ps_t)
                    nc.sync.dma_start(out=o_h[:], in_=h_nat)
                    nc.sync.dma_start(out=o_c[:], in_=c_nat)
                    return hT

                h1T = lstm_cell("l1", d_wl1ihT, d_wl1hhT,
                                [xT[:, k] for k in range(LK)], r1h, r1c, bl1,
                                o_r1h, o_r1c)
                x2T = work.tile([128, LK, BL], DT, tag="x2T")
                for k in range(LK):
                    nc.vector.tensor_add(x2T[:, k], xT[:, k], h1T[:, k])
                h2T = lstm_cell("l2", d_wl2ihT, d_wl2hhT,
                                [x2T[:, k] for k in range(LK)], r2h, r2c, bl2,
                                o_r2h, o_r2c)
                x3T = work.tile([128, LK, BL], DT, tag="x3T")
                for k in range(LK):
                    nc.vector.tensor_add(x3T[:, k], x2T[:, k], h2T[:, k])

                # mel projection (no bias), NMEL = 80*r columns
                n_mg = (NMEL + 127) // 128
                mel_nat = work.tile([BL, NMEL], DT, tag="mel_nat")
                for g in range(n_mg):
                    cols = min(128, NMEL - g * 128)
                    ps_m = pl.tile([128, BL], DT, tag="ps_m")
                    for k in range(LK):
                        nc.tensor.matmul(ps_m[:cols],
                                         wmel[:, k, g * 128:g * 128 + cols],
                                         x3T[:, k],
                                         start=(k == 0), stop=(k == LK - 1))
                    melc = work.tile([128, BL], DT, tag="melc")
                    nc.vector.tensor_copy(melc[:cols], ps_m[:cols])
                    ps_t = pt.tile([BL, 128], DT, tag="ps_tr")
                    nc.tensor.transpose(ps_t[:, :cols], melc[:cols],
                                        ident[:cols, :cols])
                    nc.vector.tensor_copy(mel_nat[:, g * 128:g * 128 + cols],
                                          ps_t[:, :cols])
                nc.sync.dma_start(out=o_mels[:], in_=mel_nat)

    nc.finalize()
    return nc


_CACHE = {}
# test harness hooks: extra kwargs for run_bass_kernel_spmd (e.g. trace=True)
# and the last BassKernelResults (for exec_time_ns). The grading harness
# calls kernel() directly and never touches these.
_RUN_KWARGS = {}
_LAST_RESULT = [None]


def _get_nc(r: int):
    if r not in _CACHE:
        _CACHE[r] = _build(r)
    return _CACHE[r]


def kernel(**inputs):
    f32 = lambda x: np.ascontiguousarray(np.asarray(x), dtype=np.float32)
    r = int(np.asarray(inputs["r"]))

    enc_full = f32(inputs["encoder_seq"])
    esp_full = f32(inputs["encoder_seq_proj"])
    prenet = f32(inputs["prenet_in"])
    attnh = f32(inputs["attn_hidden"])
    r1h_, r2h_ = f32(inputs["rnn1_hidden"]), f32(inputs["rnn2_hidden"])
    r1c_, r2c_ = f32(inputs["rnn1_cell"]), f32(inputs["rnn2_cell"])
    ctxv = f32(inputs["context_vec"])

    if r == 0:
        # degenerate: mel output empty; still must produce the rest.
        r_build = 1
    else:
        r_build = r
    nc = _get_nc(r_build)

    # ---- shared (replicated) weight prep ----
    wT = lambda w: np.ascontiguousarray(f32(w).T)
    wfc1T = np.zeros((128, 256), np.float32)
    wfc1T[:80] = f32(inputs["fc1_W"]).T
    wfc2T = wT(inputs["fc2_W"])
    wattnT = wT(inputs["attn_W"])
    wgihT = wT(inputs["gru_Wih"])
    wghhT = wT(inputs["gru_Whh"])
    wrninT = wT(inputs["rnn_in_W"])
    wl1ihT, wl1hhT = wT(inputs["l1_Wih"]), wT(inputs["l1_Whh"])
    wl2ihT, wl2hhT = wT(inputs["l2_Wih"]), wT(inputs["l2_Whh"])
    mel_W = f32(inputs["mel_W"])
    idx = (np.arange(NM)[:, None] * MAXR + np.arange(r_build)[None, :]).ravel()
    wmelT = np.ascontiguousarray(mel_W[idx].T)
    vcol = np.ascontiguousarray(f32(inputs["attn_v"]).reshape(D))
    bfc1 = f32(inputs["fc1_b"])
    bfc2 = f32(inputs["fc2_b"])
    bgih, bghh = f32(inputs["gru_bih"]), f32(inputs["gru_bhh"])
    brz = bgih[:2 * D] + bghh[:2 * D]
    bgin, bghn = bgih[2 * D:], bghh[2 * D:]
    brnin = f32(inputs["rnn_in_b"])
    bl1 = f32(inputs["l1_bih"]) + f32(inputs["l1_bhh"])
    bl2 = f32(inputs["l2_bih"]) + f32(inputs["l2_bhh"])

    shared = dict(
        wfc1T=wfc1T, wfc2T=wfc2T, wattnT=wattnT, wgihT=wgihT, wghhT=wghhT,
        wrninT=wrninT, wl1ihT=wl1ihT, wl1hhT=wl1hhT, wl2ihT=wl2ihT,
        wl2hhT=wl2hhT, wmelT=wmelT, vcol=vcol, bfc1=bfc1, bfc2=bfc2,
        brz=brz, bgin=bgin, bghn=bghn, brnin=brnin, bl1=bl1, bl2=bl2)

    in_maps = []
    for c in range(NCORES):
        s = slice(c * BL, (c + 1) * BL)
        pT = np.zeros((128, BL), np.float32)
        pT[:80] = prenet[s].T
        m = dict(shared)
        m["espT"] = np.ascontiguousarray(esp_full[s].transpose(0, 2, 1))
        m["enc"] = np.ascontiguousarray(enc_full[s])
        m["prenetT"] = pT
        m["attnhT"] = np.ascontiguousarray(attnh[s].T)
        m["ctxvT"] = np.ascontiguousarray(ctxv[s].T)
        m["r1hT"] = np.ascontiguousarray(r1h_[s].T)
        m["r2hT"] = np.ascontiguousarray(r2h_[s].T)
        m["r1cT"] = np.ascontiguousarray(r1c_[s].T)
        m["r2cT"] = np.ascontiguousarray(r2c_[s].T)
        in_maps.append(m)

    res = run_bass_kernel_spmd(nc, in_maps, core_ids=list(range(NCORES)),
                               **_RUN_KWARGS)
    _LAST_RESULT[0] = res
    rs = res.results

    cat = lambda k: np.concatenate([rs[c][k] for c in range(NCORES)], axis=0)
    mels = cat("o_mels").reshape(B, NM, r_build)[:, :, :r]
    scores_t = cat("o_scores")[:, None, :]
    return (mels, scores_t, cat("o_attnh"), cat("o_r1h"), cat("o_r2h"),
            cat("o_r1c"), cat("o_r2c"), cat("o_ctx"))


# revision 18
# speedup vs baseline: 1.4484x; 1.4484x over previous
"""Trainium2 Bass kernel for the Tacotron-style decoder step (nn_Decoder).

Strategy: data-parallel over batch. 128 rows -> 16 rows on each of 8
NeuronCores. Weights are replicated; all layout work (transposes,
padding, bias merging, mel r-slicing) happens on the host in numpy so
every device DMA is a natural, contiguous-stride load.

Matmul structure: the 16-row batch rides on the PE *stationary* side
(16-column weight loads are nearly free), while the large weight
matrices stream as the moving operand. Gates come out in natural
[16, N] layout, so LSTM/GRU state math and most outputs need no
transposes.

Per-core device program:
  1. PreNet (transposed chain) -> GRU -> q, batched over 16 rows.
  2. Bahdanau attention, one row at a time, streaming the two 1MB
     encoder slices per row: esp^T [D,T] (host pre-transposed) ->
     ScalarE tanh(x + q_d) with per-partition bias; u = v'tanh with v
     as the 1-column stationary; exp on ScalarE with accum_out giving
     the softmax sum; exp row round-trips through DRAM to produce the
     column layout the context matvec needs as stationary input.
     Softmax skips max-subtraction: |u| <= sum|v_d| ~ 10, safe in fp32.
  3. rnn_in + LSTM1 + LSTM2 + mel projection, batched over 16 rows,
     gates in natural [16, 4L] layout. mel_W pre-sliced by r on host.
"""

import numpy as np

import concourse.bacc as bacc
import concourse.bass as bass
import concourse.tile as tile
from concourse import mybir
from concourse.bass_utils import run_bass_kernel_spmd
from concourse.masks import make_identity

DT = mybir.dt.float32
B, T, D, L, NM, MAXR = 128, 1024, 256, 512, 80, 20
NCORES = 8
BL = B // NCORES  # 16 rows per core
TJ = T // 128     # 8 t-tiles
DK = D // 128     # 2 d-tiles
LK = L // 128     # 4 l-tiles


def _build(r: int):
    nc = bacc.Bacc(None, target_bir_lowering=False)

    def inp(name, shape):
        return nc.declare_dram_parameter(name, list(shape), DT, isOutput=False)

    def outp(name, shape):
        return nc.declare_dram_parameter(name, list(shape), DT, isOutput=True)

    # big streamed inputs
    d_espT = inp("espT", [BL, D, T])       # encoder_seq_proj, per-row transposed
    d_enc = inp("enc", [BL, T, D])         # encoder_seq, natural
    # small per-row state
    d_prenetT = inp("prenetT", [128, BL])  # padded 80 -> 128
    d_attnhT = inp("attnhT", [D, BL])
    d_attnh_nat = inp("attnh_nat", [BL, D])
    d_ctxvT = inp("ctxvT", [D, BL])
    d_r1hT = inp("r1hT", [L, BL])
    d_r2hT = inp("r2hT", [L, BL])
    d_r1c = inp("r1c", [BL, L])
    d_r2c = inp("r2c", [BL, L])
    # weights, host pre-transposed to [in, out]
    d_wfc1T = inp("wfc1T", [128, 256])     # padded 80 -> 128 on in-dim
    d_wfc2T = inp("wfc2T", [256, 128])
    d_wattnT = inp("wattnT", [D, D])
    d_wgihT = inp("wgihT", [D + D // 2, 3 * D])
    d_wghhT = inp("wghhT", [D, 3 * D])
    d_wrninT = inp("wrninT", [2 * D, L])
    d_wl1ihT = inp("wl1ihT", [L, 4 * L])
    d_wl1hhT = inp("wl1hhT", [L, 4 * L])
    d_wl2ihT = inp("wl2ihT", [L, 4 * L])
    d_wl2hhT = inp("wl2hhT", [L, 4 * L])
    NMEL = NM * r
    d_wmelT = inp("wmelT", [L, NMEL])
    d_vcol = inp("vcol", [D])
    # biases
    d_bfc1 = inp("bfc1", [256])
    d_bfc2 = inp("bfc2", [128])
    d_brz = inp("brz", [2 * D])            # (gru_bih+gru_bhh)[0:2D]
    d_bgin = inp("bgin", [D])              # gru_bih[2D:3D]
    d_bghn = inp("bghn", [D])              # gru_bhh[2D:3D]
    d_brnin = inp("brnin", [L])
    d_bl1 = inp("bl1", [4 * L])            # l1_bih + l1_bhh
    d_bl2 = inp("bl2", [4 * L])
    # outputs
    o_mels = outp("o_mels", [BL, NMEL])
    o_scores = outp("o_scores", [BL, T])
    o_attnh = outp("o_attnh", [BL, D])
    o_r1h = outp("o_r1h", [BL, L])
    o_r2h = outp("o_r2h", [BL, L])
    o_r1c = outp("o_r1c", [BL, L])
    o_r2c = outp("o_r2c", [BL, L])
    o_ctx = outp("o_ctx", [BL, D])
    # internal scratch (engine APs must start at partition 0, so per-row
    # [1, N] results are scattered to DRAM and reloaded in batch layout)
    d_ctxscr = nc.dram_tensor("scr_ctx", [BL, D], DT)
    d_escr = nc.dram_tensor("scr_exp", [BL, T], DT)

    AF = mybir.ActivationFunctionType

    def bc16(pool, dram_vec, n, tag):
        """Broadcast a [n] DRAM vector to an SBUF [BL, n] tile."""
        t = pool.tile([BL, n], DT, tag=tag)
        ap = dram_vec[:]
        nc.gpsimd.dma_start(
            out=t,
            in_=bass.AP(tensor=ap.tensor, offset=ap.offset,
                        ap=[[0, BL], [1, n]]))
        return t

    with tile.TileContext(nc) as tc:
        with tc.tile_pool(name="consts", bufs=1) as consts, \
             tc.tile_pool(name="states", bufs=1) as states, \
             tc.tile_pool(name="lstmw", bufs=4) as lstmw, \
             tc.tile_pool(name="espp", bufs=2) as espp, \
             tc.tile_pool(name="encp", bufs=2) as encp, \
             tc.tile_pool(name="tanhp", bufs=2) as tanhp, \
             tc.tile_pool(name="rowp", bufs=3) as rowp, \
             tc.tile_pool(name="attsm", bufs=1) as attsm, \
             tc.tile_pool(name="work", bufs=1) as work:

            # ---------------- constants / small weights ----------------
            ident = consts.tile([128, 128], DT, tag="ident")
            make_identity(nc, ident)

            wfc1 = consts.tile([128, 256], DT, tag="wfc1")
            nc.gpsimd.dma_start(out=wfc1, in_=d_wfc1T[:])
            wfc2 = consts.tile([128, 2, 128], DT, tag="wfc2")
            nc.gpsimd.dma_start(
                out=wfc2, in_=d_wfc2T.rearrange("(k p) n -> p k n", p=128))
            wattn = consts.tile([128, DK, D], DT, tag="wattn")
            nc.gpsimd.dma_start(
                out=wattn, in_=d_wattnT.rearrange("(k p) n -> p k n", p=128))
            wgih = consts.tile([128, 3, 3 * D], DT, tag="wgih")
            nc.gpsimd.dma_start(
                out=wgih, in_=d_wgihT.rearrange("(k p) n -> p k n", p=128))
            wghh = consts.tile([128, DK, 3 * D], DT, tag="wghh")
            nc.gpsimd.dma_start(
                out=wghh, in_=d_wghhT.rearrange("(k p) n -> p k n", p=128))
            wrnin = consts.tile([128, 4, L], DT, tag="wrnin")
            nc.gpsimd.dma_start(
                out=wrnin, in_=d_wrninT.rearrange("(k p) n -> p k n", p=128))
            wmel = consts.tile([128, LK, NMEL], DT, tag="wmel")
            nc.gpsimd.dma_start(
                out=wmel, in_=d_wmelT.rearrange("(k p) n -> p k n", p=128))
            vcol = consts.tile([128, DK], DT, tag="vcol")
            nc.gpsimd.dma_start(
                out=vcol, in_=d_vcol.rearrange("(k p) -> p k", p=128))

            # per-partition biases for the transposed prenet chain
            bfc1 = consts.tile([128, 2], DT, tag="bfc1")
            nc.gpsimd.dma_start(
                out=bfc1, in_=d_bfc1.rearrange("(g p) -> p g", p=128))
            bfc2 = consts.tile([128, 1], DT, tag="bfc2")
            nc.gpsimd.dma_start(
                out=bfc2, in_=d_bfc2.rearrange("(g p) -> p g", p=128))
            # natural [16, n] broadcast biases
            brz_n = bc16(consts, d_brz, 2 * D, "brz_n")
            bgin_n = bc16(consts, d_bgin, D, "bgin_n")
            bghn_n = bc16(consts, d_bghn, D, "bghn_n")
            brnin_n = bc16(consts, d_brnin, L, "brnin_n")
            bl1_n = bc16(consts, d_bl1, 4 * L, "bl1_n")
            bl2_n = bc16(consts, d_bl2, 4 * L, "bl2_n")

            prenetT = states.tile([128, BL], DT, tag="prenetT")
            nc.gpsimd.dma_start(out=prenetT, in_=d_prenetT[:])
            attnh_inT = states.tile([128, DK, BL], DT, tag="attnh_inT")
            nc.gpsimd.dma_start(
                out=attnh_inT, in_=d_attnhT.rearrange("(k p) b -> p k b", p=128))
            attnh_in_nat = states.tile([BL, D], DT, tag="attnh_in_nat")
            nc.gpsimd.dma_start(out=attnh_in_nat, in_=d_attnh_nat[:])
            ctxvT = states.tile([128, DK, BL], DT, tag="ctxvT")
            nc.gpsimd.dma_start(
                out=ctxvT, in_=d_ctxvT.rearrange("(k p) b -> p k b", p=128))
            r1hT = states.tile([128, LK, BL], DT, tag="r1hT")
            nc.gpsimd.dma_start(
                out=r1hT, in_=d_r1hT.rearrange("(k p) b -> p k b", p=128))
            r2hT = states.tile([128, LK, BL], DT, tag="r2hT")
            nc.gpsimd.dma_start(
                out=r2hT, in_=d_r2hT.rearrange("(k p) b -> p k b", p=128))
            r1c = states.tile([BL, L], DT, tag="r1c")
            nc.gpsimd.dma_start(out=r1c, in_=d_r1c[:])
            r2c = states.tile([BL, L], DT, tag="r2c")
            nc.gpsimd.dma_start(out=r2c, in_=d_r2c[:])

            # ---------------- preamble: prenet -> GRU -> q ----------------
            attnh_nat = work.tile([BL, D], DT, tag="attnh_nat")
            with tc.tile_pool(name="psum_pre", bufs=1, space="PSUM") as pp:
                # PreNet (transposed chain): p1T [256,16], p2T [128,16]
                p1 = work.tile([128, 2, BL], DT, tag="p1")
                ps = pp.tile([128, 2, BL], DT, tag="ps_p1")
                for g in range(2):
                    nc.tensor.matmul(ps[:, g], wfc1[:, bass.ts(g, 128)],
                                     prenetT, start=True, stop=True)
                for g in range(2):
                    nc.scalar.activation(p1[:, g], ps[:, g], AF.Relu,
                                         bias=bfc1[:, g:g + 1])
                p2 = work.tile([128, BL], DT, tag="p2")
                ps2 = pp.tile([128, BL], DT, tag="ps_p2")
                for k in range(2):
                    nc.tensor.matmul(ps2, wfc2[:, k], p1[:, k],
                                     start=(k == 0), stop=(k == 1))
                nc.scalar.activation(p2, ps2, AF.Relu, bias=bfc2[:, 0:1])

                # GRU, natural [16, 768] gates; x_in = [ctxv; p2]
                # chunk at 512-f32 PSUM bank boundaries: [0:512], [512:768]
                gchunks = [(0, 512), (512, 256)]
                ps_gi = pp.tile([BL, 3 * D], DT, tag="ps_gi")
                xin = [ctxvT[:, 0], ctxvT[:, 1], p2]
                for k in range(3):
                    for c0, cw in gchunks:
                        nc.tensor.matmul(ps_gi[:, c0:c0 + cw], xin[k],
                                         wgih[:, k, c0:c0 + cw],
                                         start=(k == 0), stop=(k == 2))
                ps_gh = pp.tile([BL, 3 * D], DT, tag="ps_gh")
                for k in range(DK):
                    for c0, cw in gchunks:
                        nc.tensor.matmul(ps_gh[:, c0:c0 + cw],
                                         attnh_inT[:, k],
                                         wghh[:, k, c0:c0 + cw],
                                         start=(k == 0), stop=(k == DK - 1))
                gh = work.tile([BL, 3 * D], DT, tag="gh")
                nc.vector.tensor_copy(gh, ps_gh)

                t_r = work.tile([BL, D], DT, tag="t_r")
                rg = work.tile([BL, D], DT, tag="rg")
                zg = work.tile([BL, D], DT, tag="zg")
                ng = work.tile([BL, D], DT, tag="ng")
                nc.vector.tensor_add(t_r, ps_gi[:, :D], gh[:, :D])
                nc.vector.tensor_add(t_r, t_r, brz_n[:, :D])
                nc.scalar.activation(rg, t_r, AF.Sigmoid)
                nc.vector.tensor_add(t_r, ps_gi[:, D:2 * D], gh[:, D:2 * D])
                nc.vector.tensor_add(t_r, t_r, brz_n[:, D:])
                nc.scalar.activation(zg, t_r, AF.Sigmoid)
                nc.vector.tensor_add(t_r, gh[:, 2 * D:], bghn_n)
                nc.vector.tensor_mul(t_r, rg, t_r)
                nc.vector.tensor_add(t_r, ps_gi[:, 2 * D:], t_r)
                nc.vector.tensor_add(t_r, t_r, bgin_n)
                nc.scalar.activation(ng, t_r, AF.Tanh)
                # h' = ng + zg*(h - ng)
                nc.vector.tensor_sub(t_r, attnh_in_nat, ng)
                nc.vector.tensor_mul(t_r, zg, t_r)
                nc.vector.tensor_add(attnh_nat, ng, t_r)
                nc.sync.dma_start(out=o_attnh[:], in_=attnh_nat)

            # transposes: attn_h -> [D, 16]; q -> [D, 16]
            attnhT = work.tile([128, DK, BL], DT, tag="attnhT")
            qT = work.tile([128, DK, BL], DT, tag="qT")
            with tc.tile_pool(name="psum_tq", bufs=2, space="PSUM") as pt, \
                 tc.tile_pool(name="psum_q", bufs=1, space="PSUM") as pq:
                for k in range(DK):
                    ps_t = pt.tile([128, BL], DT, tag="ps_tq")
                    nc.tensor.transpose(ps_t, attnh_nat[:, bass.ts(k, 128)],
                                        ident[:BL, :BL])
                    nc.vector.tensor_copy(attnhT[:, k], ps_t)
                ps_qn = pq.tile([BL, D], DT, tag="ps_qn")
                for k in range(DK):
                    nc.tensor.matmul(ps_qn, attnhT[:, k], wattn[:, k],
                                     start=(k == 0), stop=(k == DK - 1))
                q_nat = work.tile([BL, D], DT, tag="q_nat")
                nc.vector.tensor_copy(q_nat, ps_qn)
                for k in range(DK):
                    ps_t = pt.tile([128, BL], DT, tag="ps_tq")
                    nc.tensor.transpose(ps_t, q_nat[:, bass.ts(k, 128)],
                                        ident[:BL, :BL])
                    nc.vector.tensor_copy(qT[:, k], ps_t)

            # ---------------- attention: stream rows ----------------
            S_row = attsm.tile([1, BL], DT, tag="S_row")
            with tc.tile_pool(name="psum_u", bufs=2, space="PSUM") as pu, \
                 tc.tile_pool(name="psum_c", bufs=3, space="PSUM") as pc:
                for b in range(BL):
                    esp = espp.tile([128, DK, T], DT, tag="esp")
                    nc.sync.dma_start(
                        out=esp,
                        in_=d_espT[b].rearrange("(k p) t -> p k t", p=128))
                    enc = encp.tile([128, TJ, D], DT, tag="enc")
                    nc.sync.dma_start(
                        out=enc,
                        in_=d_enc[b].rearrange("(j p) d -> p j d", p=128))

                    th = tanhp.tile([128, DK, T], DT, tag="th")
                    for k in range(DK):
                        nc.scalar.activation(th[:, k], esp[:, k], AF.Tanh,
                                             bias=qT[:, k, b:b + 1])
                    # u[t] = sum_d v_d * tanh -> [1, T] psum (2 banks)
                    ps_u = pu.tile([1, T], DT, tag="ps_u")
                    for h in range(2):
                        for k in range(DK):
                            nc.tensor.matmul(ps_u[:, bass.ts(h, 512)],
                                             vcol[:, k:k + 1],
                                             th[:, k, bass.ts(h, 512)],
                                             start=(k == 0), stop=(k == DK - 1))
                    # exp + total row sum; row round-trips through DRAM
                    exp_nat = rowp.tile([1, T], DT, tag="exp_nat")
                    nc.scalar.activation(exp_nat, ps_u, AF.Exp,
                                         accum_out=S_row[:, b:b + 1])
                    nc.sync.dma_start(out=d_escr[b:b + 1, :], in_=exp_nat)
                    expT = rowp.tile([128, TJ], DT, tag="expT")
                    nc.sync.dma_start(
                        out=expT,
                        in_=d_escr[b].rearrange("(j p) -> p j", p=128))
                    # context = exp_u @ enc
                    ps_ctx = pc.tile([1, D], DT, tag="ps_ctx")
                    for j in range(TJ):
                        nc.tensor.matmul(ps_ctx, expT[:, j:j + 1], enc[:, j],
                                         start=(j == 0), stop=(j == TJ - 1))
                    ctmp = rowp.tile([1, D], DT, tag="ctmp")
                    nc.scalar.activation(ctmp, ps_ctx, AF.Copy)
                    nc.sync.dma_start(out=d_ctxscr[b:b + 1, :], in_=ctmp)

            # ---------------- softmax normalization ----------------
            ctxn = attsm.tile([BL, D], DT, tag="ctxn")
            nc.sync.dma_start(out=ctxn, in_=d_ctxscr[:])
            scores = attsm.tile([BL, T], DT, tag="scores")
            nc.sync.dma_start(out=scores, in_=d_escr[:])
            ctxT = work.tile([128, DK, BL], DT, tag="ctxT")
            with tc.tile_pool(name="psum_n", bufs=2, space="PSUM") as pn:
                ps_sc = pn.tile([BL, 1], DT, tag="ps_sc")
                nc.tensor.transpose(ps_sc, S_row, ident[:1, :1])
                rcol = attsm.tile([BL, 1], DT, tag="rcol")
                nc.vector.reciprocal(rcol, ps_sc)
                nc.vector.tensor_scalar_mul(ctxn, ctxn, rcol)
                nc.sync.dma_start(out=o_ctx[:], in_=ctxn)
                nc.vector.tensor_scalar_mul(scores, scores, rcol)
                nc.sync.dma_start(out=o_scores[:], in_=scores)
                for k in range(DK):
                    ps_t = pn.tile([128, BL], DT, tag="ps_tc")
                    nc.tensor.transpose(ps_t, ctxn[:, bass.ts(k, 128)],
                                        ident[:BL, :BL])
                    nc.vector.tensor_copy(ctxT[:, k], ps_t)

            # ---------------- rnn_in + LSTMs + mel ----------------
            with tc.tile_pool(name="psum_l", bufs=1, space="PSUM") as pl, \
                 tc.tile_pool(name="psum_t2", bufs=2, space="PSUM") as pt:

                def transpose_nat(src_nat, tag):
                    """[16, 512] natural -> [128, LK, 16] column tiles."""
                    dst = work.tile([128, LK, BL], DT, tag=tag)
                    for k in range(LK):
                        ps_t = pt.tile([128, BL], DT, tag="ps_tr")
                        nc.tensor.transpose(ps_t, src_nat[:, bass.ts(k, 128)],
                                            ident[:BL, :BL])
                        nc.vector.tensor_copy(dst[:, k], ps_t)
                    return dst

                ps_x = pl.tile([BL, L], DT, tag="ps_x")
                xin2 = [ctxT[:, 0], ctxT[:, 1], attnhT[:, 0], attnhT[:, 1]]
                for k in range(4):
                    nc.tensor.matmul(ps_x, xin2[k], wrnin[:, k],
                                     start=(k == 0), stop=(k == 3))
                x_nat = work.tile([BL, L], DT, tag="x_nat")
                nc.vector.tensor_add(x_nat, ps_x, brnin_n)
                xT = transpose_nat(x_nat, "xT")

                def lstm_cell(tag, d_wih, d_whh, xT_t, hT_t, c_nat, bl_n,
                              o_h, o_c):
                    # stream weights per k-tile: [128, 2048] (1 MB) chunks
                    ps_g = pl.tile([BL, 4 * L], DT, tag="ps_g")
                    for phase, (d_w, lhs) in enumerate(
                            [(d_wih, xT_t), (d_whh, hT_t)]):
                        wv = d_w.rearrange("(k p) n -> k p n", p=128)
                        for k in range(LK):
                            w = lstmw.tile([128, 4 * L], DT, tag="wlstm")
                            nc.gpsimd.dma_start(out=w, in_=wv[k])
                            for c in range(4):
                                nc.tensor.matmul(
                                    ps_g[:, bass.ts(c, 512)], lhs[:, k],
                                    w[:, bass.ts(c, 512)],
                                    start=(phase == 0 and k == 0),
                                    stop=(phase == 1 and k == LK - 1))
                    si = work.tile([BL, L], DT, tag="si")
                    sf = work.tile([BL, L], DT, tag="sf")
                    tg = work.tile([BL, L], DT, tag="tg")
                    so = work.tile([BL, L], DT, tag="so")
                    tb = work.tile([BL, L], DT, tag="tb")
                    nc.vector.tensor_add(tb, ps_g[:, :L], bl_n[:, :L])
                    nc.scalar.activation(si, tb, AF.Sigmoid)
                    nc.vector.tensor_add(tb, ps_g[:, L:2 * L], bl_n[:, L:2 * L])
                    nc.scalar.activation(sf, tb, AF.Sigmoid)
                    nc.vector.tensor_add(tb, ps_g[:, 2 * L:3 * L],
                                         bl_n[:, 2 * L:3 * L])
                    nc.scalar.activation(tg, tb, AF.Tanh)
                    nc.vector.tensor_add(tb, ps_g[:, 3 * L:], bl_n[:, 3 * L:])
                    nc.scalar.activation(so, tb, AF.Sigmoid)
                    c_new = work.tile([BL, L], DT, tag=f"c_new_{tag}")
                    nc.vector.tensor_mul(sf, sf, c_nat)
                    nc.vector.tensor_mul(si, si, tg)
                    nc.vector.tensor_add(c_new, sf, si)
                    nc.sync.dma_start(out=o_c[:], in_=c_new)
                    h_new = work.tile([BL, L], DT, tag=f"h_new_{tag}")
                    nc.scalar.activation(tg, c_new, AF.Tanh)
                    nc.vector.tensor_mul(h_new, so, tg)
                    nc.sync.dma_start(out=o_h[:], in_=h_new)
                    return h_new

                h1 = lstm_cell("l1", d_wl1ihT, d_wl1hhT, xT, r1hT, r1c, bl1_n,
                               o_r1h, o_r1c)
                x2 = work.tile([BL, L], DT, tag="x2")
                nc.vector.tensor_add(x2, x_nat, h1)
                x2T = transpose_nat(x2, "x2T")
                h2 = lstm_cell("l2", d_wl2ihT, d_wl2hhT, x2T, r2hT, r2c, bl2_n,
                               o_r2h, o_r2c)
                x3 = work.tile([BL, L], DT, tag="x3")
                nc.vector.tensor_add(x3, x2, h2)
                x3T = transpose_nat(x3, "x3T")

                # mel projection (no bias), NMEL = 80*r columns
                ps_m = pl.tile([BL, NMEL], DT, tag="ps_m")
                for k in range(LK):
                    nc.tensor.matmul(ps_m, x3T[:, k], wmel[:, k],
                                     start=(k == 0), stop=(k == LK - 1))
                mel_nat = work.tile([BL, NMEL], DT, tag="mel_nat")
                nc.vector.tensor_copy(mel_nat, ps_m)
                nc.sync.dma_start(out=o_mels[:], in_=mel_nat)

    nc.finalize()
    return nc


_CACHE = {}
# test harness hooks: extra kwargs for run_bass_kernel_spmd (e.g. trace=True)
# and the last BassKernelResults (for exec_time_ns). The grading harness
# calls kernel() directly and never touches these.
_RUN_KWARGS = {}
_LAST_RESULT = [None]


def _get_nc(r: int):
    if r not in _CACHE:
        _CACHE[r] = _build(r)
    return _CACHE[r]


def kernel(**inputs):
    f32 = lambda x: np.ascontiguousarray(np.asarray(x), dtype=np.float32)
    r = int(np.asarray(inputs["r"]))

    enc_full = f32(inputs["encoder_seq"])
    esp_full = f32(inputs["encoder_seq_proj"])
    prenet = f32(inputs["prenet_in"])
    attnh = f32(inputs["attn_hidden"])
    r1h_, r2h_ = f32(inputs["rnn1_hidden"]), f32(inputs["rnn2_hidden"])
    r1c_, r2c_ = f32(inputs["rnn1_cell"]), f32(inputs["rnn2_cell"])
    ctxv = f32(inputs["context_vec"])

    r_build = 1 if r == 0 else r
    nc = _get_nc(r_build)

    # ---- shared (replicated) weight prep ----
    wT = lambda w: np.ascontiguousarray(f32(w).T)
    wfc1T = np.zeros((128, 256), np.float32)
    wfc1T[:80] = f32(inputs["fc1_W"]).T
    wfc2T = wT(inputs["fc2_W"])
    wattnT = wT(inputs["attn_W"])
    wgihT = wT(inputs["gru_Wih"])
    wghhT = wT(inputs["gru_Whh"])
    wrninT = wT(inputs["rnn_in_W"])
    wl1ihT, wl1hhT = wT(inputs["l1_Wih"]), wT(inputs["l1_Whh"])
    wl2ihT, wl2hhT = wT(inputs["l2_Wih"]), wT(inputs["l2_Whh"])
    mel_W = f32(inputs["mel_W"])
    idx = (np.arange(NM)[:, None] * MAXR + np.arange(r_build)[None, :]).ravel()
    wmelT = np.ascontiguousarray(mel_W[idx].T)
    vcol = np.ascontiguousarray(f32(inputs["attn_v"]).reshape(D))
    bfc1 = f32(inputs["fc1_b"])
    bfc2 = f32(inputs["fc2_b"])
    bgih, bghh = f32(inputs["gru_bih"]), f32(inputs["gru_bhh"])
    brz = bgih[:2 * D] + bghh[:2 * D]
    bgin, bghn = bgih[2 * D:], bghh[2 * D:]
    brnin = f32(inputs["rnn_in_b"])
    bl1 = f32(inputs["l1_bih"]) + f32(inputs["l1_bhh"])
    bl2 = f32(inputs["l2_bih"]) + f32(inputs["l2_bhh"])

    shared = dict(
        wfc1T=wfc1T, wfc2T=wfc2T, wattnT=wattnT, wgihT=wgihT, wghhT=wghhT,
        wrninT=wrninT, wl1ihT=wl1ihT, wl1hhT=wl1hhT, wl2ihT=wl2ihT,
        wl2hhT=wl2hhT, wmelT=wmelT, vcol=vcol, bfc1=bfc1, bfc2=bfc2,
        brz=brz, bgin=bgin, bghn=bghn, brnin=brnin, bl1=bl1, bl2=bl2)

    in_maps = []
    for c in range(NCORES):
        s = slice(c * BL, (c + 1) * BL)
        pT = np.zeros((128, BL), np.float32)
        pT[:80] = prenet[s].T
        m = dict(shared)
        m["espT"] = np.ascontiguousarray(esp_full[s].transpose(0, 2, 1))
        m["enc"] = np.ascontiguousarray(enc_full[s])
        m["prenetT"] = pT
        m["attnhT"] = np.ascontiguousarray(attnh[s].T)
        m["attnh_nat"] = np.ascontiguousarray(attnh[s])
        m["ctxvT"] = np.ascontiguousarray(ctxv[s].T)
        m["r1hT"] = np.ascontiguousarray(r1h_[s].T)
        m["r2hT"] = np.ascontiguousarray(r2h_[s].T)
        m["r1c"] = np.ascontiguousarray(r1c_[s])
        m["r2c"] = np.ascontiguousarray(r2c_[s])
        in_maps.append(m)

    res = run_bass_kernel_spmd(nc, in_maps, core_ids=list(range(NCORES)),
                               **_RUN_KWARGS)
    _LAST_RESULT[0] = res
    rs = res.results

    cat = lambda k: np.concatenate([rs[c][k] for c in range(NCORES)], axis=0)
    mels = cat("o_mels").reshape(B, NM, r_build)[:, :, :r]
    scores_t = cat("o_scores")[:, None, :]
    return (mels, scores_t, cat("o_attnh"), cat("o_r1h"), cat("o_r2h"),
            cat("o_r1c"), cat("o_r2c"), cat("o_ctx"))


# revision 20
# speedup vs baseline: 1.9692x; 1.3595x over previous
"""Trainium2 Bass kernel for the Tacotron-style decoder step (nn_Decoder).

Strategy: data-parallel over batch. 128 rows -> 16 rows on each of 8
NeuronCores. Weights are replicated; all layout work (transposes,
padding, bias merging, mel r-slicing) happens on the host in numpy so
every device DMA is a natural, contiguous-stride load.

Matmul structure: the 16-row batch rides on the PE *stationary* side
(16-column weight loads are nearly free), while the large weight
matrices stream as the moving operand. Gates come out in natural
[16, N] layout, so LSTM/GRU state math and most outputs need no
transposes.

Per-core device program:
  1. PreNet (transposed chain) -> GRU -> q, batched over 16 rows.
  2. Bahdanau attention, one row at a time, streaming the two 1MB
     encoder slices per row: esp^T [D,T] (host pre-transposed) ->
     ScalarE tanh(x + q_d) with per-partition bias; u = v'tanh with v
     as the 1-column stationary; exp on ScalarE with accum_out giving
     the softmax sum; exp row round-trips through DRAM to produce the
     column layout the context matvec needs as stationary input.
     Softmax skips max-subtraction: |u| <= sum|v_d| ~ 10, safe in fp32.
  3. rnn_in + LSTM1 + LSTM2 + mel projection, batched over 16 rows,
     gates in natural [16, 4L] layout. mel_W pre-sliced by r on host.
"""

import ml_dtypes
import numpy as np

import concourse.bacc as bacc
import concourse.bass as bass
import concourse.tile as tile
from concourse import mybir
from concourse.bass_utils import run_bass_kernel_spmd
from concourse.masks import make_identity

DT = mybir.dt.float32
# bf16 for the heavy PE streams: fp32 matmuls are double-pumped (FP32
# HI/LO passes) on the PE and cost 2x; bf16 also halves the dominant
# DMA traffic (encoder tensors + LSTM weights). PSUM accumulation is
# fp32 regardless; all state/elementwise math stays fp32.
BF = mybir.dt.bfloat16
B, T, D, L, NM, MAXR = 128, 1024, 256, 512, 80, 20
NCORES = 8
BL = B // NCORES  # 16 rows per core
TJ = T // 128     # 8 t-tiles
DK = D // 128     # 2 d-tiles
LK = L // 128     # 4 l-tiles


def _build(r: int):
    nc = bacc.Bacc(None, target_bir_lowering=False)

    def inp(name, shape, dt=DT):
        return nc.declare_dram_parameter(name, list(shape), dt, isOutput=False)

    def outp(name, shape):
        return nc.declare_dram_parameter(name, list(shape), DT, isOutput=True)

    # big streamed inputs
    d_espT = inp("espT", [BL, D, T], BF)       # encoder_seq_proj, per-row transposed
    d_enc = inp("enc", [BL, T, D], BF)         # encoder_seq, natural
    # small per-row state
    d_prenetT = inp("prenetT", [128, BL])  # padded 80 -> 128
    d_attnhT = inp("attnhT", [D, BL])
    d_attnh_nat = inp("attnh_nat", [BL, D])
    d_ctxvT = inp("ctxvT", [D, BL])
    d_r1hT = inp("r1hT", [L, BL], BF)
    d_r2hT = inp("r2hT", [L, BL], BF)
    d_r1c = inp("r1c", [BL, L])
    d_r2c = inp("r2c", [BL, L])
    # weights, host pre-transposed to [in, out]
    d_wfc1T = inp("wfc1T", [128, 256])     # padded 80 -> 128 on in-dim
    d_wfc2T = inp("wfc2T", [256, 128])
    d_wattnT = inp("wattnT", [D, D])
    d_wgihT = inp("wgihT", [D + D // 2, 3 * D])
    d_wghhT = inp("wghhT", [D, 3 * D])
    d_wrninT = inp("wrninT", [2 * D, L])
    d_wl1ihT = inp("wl1ihT", [L, 4 * L], BF)
    d_wl1hhT = inp("wl1hhT", [L, 4 * L], BF)
    d_wl2ihT = inp("wl2ihT", [L, 4 * L], BF)
    d_wl2hhT = inp("wl2hhT", [L, 4 * L], BF)
    NMEL = NM * r
    d_wmelT = inp("wmelT", [L, NMEL])
    d_vcol = inp("vcol", [D], BF)
    # biases
    d_bfc1 = inp("bfc1", [256])
    d_bfc2 = inp("bfc2", [128])
    d_brz = inp("brz", [2 * D])            # (gru_bih+gru_bhh)[0:2D]
    d_bgin = inp("bgin", [D])              # gru_bih[2D:3D]
    d_bghn = inp("bghn", [D])              # gru_bhh[2D:3D]
    d_brnin = inp("brnin", [L])
    d_bl1 = inp("bl1", [4 * L])            # l1_bih + l1_bhh
    d_bl2 = inp("bl2", [4 * L])
    # outputs
    o_mels = outp("o_mels", [BL, NMEL])
    o_scores = outp("o_scores", [BL, T])
    o_attnh = outp("o_attnh", [BL, D])
    o_r1h = outp("o_r1h", [BL, L])
    o_r2h = outp("o_r2h", [BL, L])
    o_r1c = outp("o_r1c", [BL, L])
    o_r2c = outp("o_r2c", [BL, L])
    o_ctx = outp("o_ctx", [BL, D])
    # internal scratch (engine APs must start at partition 0, so per-row
    # [1, N] results are scattered to DRAM and reloaded in batch layout)
    d_ctxscr = nc.dram_tensor("scr_ctx", [BL, D], DT)
    d_escr = nc.dram_tensor("scr_exp", [BL, T], DT)

    AF = mybir.ActivationFunctionType

    def bc16(pool, dram_vec, n, tag):
        """Broadcast a [n] DRAM vector to an SBUF [BL, n] tile."""
        t = pool.tile([BL, n], DT, tag=tag)
        ap = dram_vec[:]
        nc.gpsimd.dma_start(
            out=t,
            in_=bass.AP(tensor=ap.tensor, offset=ap.offset,
                        ap=[[0, BL], [1, n]]))
        return t

    with tile.TileContext(nc) as tc:
        with tc.tile_pool(name="consts", bufs=1) as consts, \
             tc.tile_pool(name="states", bufs=1) as states, \
             tc.tile_pool(name="lstmw", bufs=4) as lstmw, \
             tc.tile_pool(name="espp", bufs=2) as espp, \
             tc.tile_pool(name="encp", bufs=2) as encp, \
             tc.tile_pool(name="tanhp", bufs=2) as tanhp, \
             tc.tile_pool(name="rowp", bufs=3) as rowp, \
             tc.tile_pool(name="attsm", bufs=1) as attsm, \
             tc.tile_pool(name="work", bufs=1) as work:

            # ---------------- constants / small weights ----------------
            ident = consts.tile([128, 128], DT, tag="ident")
            make_identity(nc, ident)

            wfc1 = consts.tile([128, 256], DT, tag="wfc1")
            nc.gpsimd.dma_start(out=wfc1, in_=d_wfc1T[:])
            wfc2 = consts.tile([128, 2, 128], DT, tag="wfc2")
            nc.gpsimd.dma_start(
                out=wfc2, in_=d_wfc2T.rearrange("(k p) n -> p k n", p=128))
            wattn = consts.tile([128, DK, D], DT, tag="wattn")
            nc.gpsimd.dma_start(
                out=wattn, in_=d_wattnT.rearrange("(k p) n -> p k n", p=128))
            wgih = consts.tile([128, 3, 3 * D], DT, tag="wgih")
            nc.gpsimd.dma_start(
                out=wgih, in_=d_wgihT.rearrange("(k p) n -> p k n", p=128))
            wghh = consts.tile([128, DK, 3 * D], DT, tag="wghh")
            nc.gpsimd.dma_start(
                out=wghh, in_=d_wghhT.rearrange("(k p) n -> p k n", p=128))
            wrnin = consts.tile([128, 4, L], DT, tag="wrnin")
            nc.gpsimd.dma_start(
                out=wrnin, in_=d_wrninT.rearrange("(k p) n -> p k n", p=128))
            wmel = consts.tile([128, LK, NMEL], DT, tag="wmel")
            nc.gpsimd.dma_start(
                out=wmel, in_=d_wmelT.rearrange("(k p) n -> p k n", p=128))
            vcol = consts.tile([128, DK], BF, tag="vcol")
            nc.gpsimd.dma_start(
                out=vcol, in_=d_vcol.rearrange("(k p) -> p k", p=128))

            # per-partition biases for the transposed prenet chain
            bfc1 = consts.tile([128, 2], DT, tag="bfc1")
            nc.gpsimd.dma_start(
                out=bfc1, in_=d_bfc1.rearrange("(g p) -> p g", p=128))
            bfc2 = consts.tile([128, 1], DT, tag="bfc2")
            nc.gpsimd.dma_start(
                out=bfc2, in_=d_bfc2.rearrange("(g p) -> p g", p=128))
            # natural [16, n] broadcast biases
            brz_n = bc16(consts, d_brz, 2 * D, "brz_n")
            bgin_n = bc16(consts, d_bgin, D, "bgin_n")
            bghn_n = bc16(consts, d_bghn, D, "bghn_n")
            brnin_n = bc16(consts, d_brnin, L, "brnin_n")
            bl1_n = bc16(consts, d_bl1, 4 * L, "bl1_n")
            bl2_n = bc16(consts, d_bl2, 4 * L, "bl2_n")

            prenetT = states.tile([128, BL], DT, tag="prenetT")
            nc.gpsimd.dma_start(out=prenetT, in_=d_prenetT[:])
            attnh_inT = states.tile([128, DK, BL], DT, tag="attnh_inT")
            nc.gpsimd.dma_start(
                out=attnh_inT, in_=d_attnhT.rearrange("(k p) b -> p k b", p=128))
            attnh_in_nat = states.tile([BL, D], DT, tag="attnh_in_nat")
            nc.gpsimd.dma_start(out=attnh_in_nat, in_=d_attnh_nat[:])
            ctxvT = states.tile([128, DK, BL], DT, tag="ctxvT")
            nc.gpsimd.dma_start(
                out=ctxvT, in_=d_ctxvT.rearrange("(k p) b -> p k b", p=128))
            r1hT = states.tile([128, LK, BL], BF, tag="r1hT")
            nc.gpsimd.dma_start(
                out=r1hT, in_=d_r1hT.rearrange("(k p) b -> p k b", p=128))
            r2hT = states.tile([128, LK, BL], BF, tag="r2hT")
            nc.gpsimd.dma_start(
                out=r2hT, in_=d_r2hT.rearrange("(k p) b -> p k b", p=128))
            r1c = states.tile([BL, L], DT, tag="r1c")
            nc.gpsimd.dma_start(out=r1c, in_=d_r1c[:])
            r2c = states.tile([BL, L], DT, tag="r2c")
            nc.gpsimd.dma_start(out=r2c, in_=d_r2c[:])

            # ---------------- preamble: prenet -> GRU -> q ----------------
            attnh_nat = work.tile([BL, D], DT, tag="attnh_nat")
            with tc.tile_pool(name="psum_pre", bufs=1, space="PSUM") as pp:
                # PreNet (transposed chain): p1T [256,16], p2T [128,16]
                p1 = work.tile([128, 2, BL], DT, tag="p1")
                ps = pp.tile([128, 2, BL], DT, tag="ps_p1")
                for g in range(2):
                    nc.tensor.matmul(ps[:, g], wfc1[:, bass.ts(g, 128)],
                                     prenetT, start=True, stop=True)
                for g in range(2):
                    nc.scalar.activation(p1[:, g], ps[:, g], AF.Relu,
                                         bias=bfc1[:, g:g + 1])
                p2 = work.tile([128, BL], DT, tag="p2")
                ps2 = pp.tile([128, BL], DT, tag="ps_p2")
                for k in range(2):
                    nc.tensor.matmul(ps2, wfc2[:, k], p1[:, k],
                                     start=(k == 0), stop=(k == 1))
                nc.scalar.activation(p2, ps2, AF.Relu, bias=bfc2[:, 0:1])

                # GRU, natural [16, 768] gates; x_in = [ctxv; p2]
                # chunk at 512-f32 PSUM bank boundaries: [0:512], [512:768]
                gchunks = [(0, 512), (512, 256)]
                ps_gi = pp.tile([BL, 3 * D], DT, tag="ps_gi")
                xin = [ctxvT[:, 0], ctxvT[:, 1], p2]
                for k in range(3):
                    for c0, cw in gchunks:
                        nc.tensor.matmul(ps_gi[:, c0:c0 + cw], xin[k],
                                         wgih[:, k, c0:c0 + cw],
                                         start=(k == 0), stop=(k == 2))
                ps_gh = pp.tile([BL, 3 * D], DT, tag="ps_gh")
                for k in range(DK):
                    for c0, cw in gchunks:
                        nc.tensor.matmul(ps_gh[:, c0:c0 + cw],
                                         attnh_inT[:, k],
                                         wghh[:, k, c0:c0 + cw],
                                         start=(k == 0), stop=(k == DK - 1))
                gh = work.tile([BL, 3 * D], DT, tag="gh")
                nc.vector.tensor_copy(gh, ps_gh)

                t_r = work.tile([BL, D], DT, tag="t_r")
                rg = work.tile([BL, D], DT, tag="rg")
                zg = work.tile([BL, D], DT, tag="zg")
                ng = work.tile([BL, D], DT, tag="ng")
                nc.vector.tensor_add(t_r, ps_gi[:, :D], gh[:, :D])
                nc.vector.tensor_add(t_r, t_r, brz_n[:, :D])
                nc.scalar.activation(rg, t_r, AF.Sigmoid)
                nc.vector.tensor_add(t_r, ps_gi[:, D:2 * D], gh[:, D:2 * D])
                nc.vector.tensor_add(t_r, t_r, brz_n[:, D:])
                nc.scalar.activation(zg, t_r, AF.Sigmoid)
                nc.vector.tensor_add(t_r, gh[:, 2 * D:], bghn_n)
                nc.vector.tensor_mul(t_r, rg, t_r)
                nc.vector.tensor_add(t_r, ps_gi[:, 2 * D:], t_r)
                nc.vector.tensor_add(t_r, t_r, bgin_n)
                nc.scalar.activation(ng, t_r, AF.Tanh)
                # h' = ng + zg*(h - ng)
                nc.vector.tensor_sub(t_r, attnh_in_nat, ng)
                nc.vector.tensor_mul(t_r, zg, t_r)
                nc.vector.tensor_add(attnh_nat, ng, t_r)
                nc.sync.dma_start(out=o_attnh[:], in_=attnh_nat)

            # transposes: attn_h -> [D, 16]; q -> [D, 16]
            attnhT = work.tile([128, DK, BL], DT, tag="attnhT")
            qT = work.tile([128, DK, BL], DT, tag="qT")
            with tc.tile_pool(name="psum_tq", bufs=2, space="PSUM") as pt, \
                 tc.tile_pool(name="psum_q", bufs=1, space="PSUM") as pq:
                for k in range(DK):
                    ps_t = pt.tile([128, BL], DT, tag="ps_tq")
                    nc.tensor.transpose(ps_t, attnh_nat[:, bass.ts(k, 128)],
                                        ident[:BL, :BL])
                    nc.vector.tensor_copy(attnhT[:, k], ps_t)
                ps_qn = pq.tile([BL, D], DT, tag="ps_qn")
                for k in range(DK):
                    nc.tensor.matmul(ps_qn, attnhT[:, k], wattn[:, k],
                                     start=(k == 0), stop=(k == DK - 1))
                q_nat = work.tile([BL, D], DT, tag="q_nat")
                nc.vector.tensor_copy(q_nat, ps_qn)
                for k in range(DK):
                    ps_t = pt.tile([128, BL], DT, tag="ps_tq")
                    nc.tensor.transpose(ps_t, q_nat[:, bass.ts(k, 128)],
                                        ident[:BL, :BL])
                    nc.vector.tensor_copy(qT[:, k], ps_t)

            # ---------------- attention: stream rows ----------------
            S_row = attsm.tile([1, BL], DT, tag="S_row")
            with tc.tile_pool(name="psum_u", bufs=2, space="PSUM") as pu, \
                 tc.tile_pool(name="psum_c", bufs=3, space="PSUM") as pc:
                for b in range(BL):
                    esp = espp.tile([128, DK, T], BF, tag="esp")
                    nc.sync.dma_start(
                        out=esp,
                        in_=d_espT[b].rearrange("(k p) t -> p k t", p=128))
                    enc = encp.tile([128, TJ, D], BF, tag="enc")
                    nc.sync.dma_start(
                        out=enc,
                        in_=d_enc[b].rearrange("(j p) d -> p j d", p=128))

                    th = tanhp.tile([128, DK, T], BF, tag="th")
                    for k in range(DK):
                        nc.scalar.activation(th[:, k], esp[:, k], AF.Tanh,
                                             bias=qT[:, k, b:b + 1])
                    # u[t] = sum_d v_d * tanh -> [1, T] psum (2 banks)
                    ps_u = pu.tile([1, T], DT, tag="ps_u")
                    for h in range(2):
                        for k in range(DK):
                            nc.tensor.matmul(ps_u[:, bass.ts(h, 512)],
                                             vcol[:, k:k + 1],
                                             th[:, k, bass.ts(h, 512)],
                                             start=(k == 0), stop=(k == DK - 1))
                    # exp + total row sum; row round-trips through DRAM
                    exp_nat = rowp.tile([1, T], DT, tag="exp_nat")
                    nc.scalar.activation(exp_nat, ps_u, AF.Exp,
                                         accum_out=S_row[:, b:b + 1])
                    nc.sync.dma_start(out=d_escr[b:b + 1, :], in_=exp_nat)
                    expT = rowp.tile([128, TJ], BF, tag="expT")
                    nc.gpsimd.dma_start(
                        out=expT,
                        in_=d_escr[b].rearrange("(j p) -> p j", p=128))
                    # context = exp_u @ enc
                    ps_ctx = pc.tile([1, D], DT, tag="ps_ctx")
                    for j in range(TJ):
                        nc.tensor.matmul(ps_ctx, expT[:, j:j + 1], enc[:, j],
                                         start=(j == 0), stop=(j == TJ - 1))
                    ctmp = rowp.tile([1, D], DT, tag="ctmp")
                    nc.scalar.activation(ctmp, ps_ctx, AF.Copy)
                    nc.sync.dma_start(out=d_ctxscr[b:b + 1, :], in_=ctmp)

            # ---------------- softmax normalization ----------------
            ctxn = attsm.tile([BL, D], DT, tag="ctxn")
            nc.sync.dma_start(out=ctxn, in_=d_ctxscr[:])
            scores = attsm.tile([BL, T], DT, tag="scores")
            nc.sync.dma_start(out=scores, in_=d_escr[:])
            ctxT = work.tile([128, DK, BL], DT, tag="ctxT")
            with tc.tile_pool(name="psum_n", bufs=2, space="PSUM") as pn:
                ps_sc = pn.tile([BL, 1], DT, tag="ps_sc")
                nc.tensor.transpose(ps_sc, S_row, ident[:1, :1])
                rcol = attsm.tile([BL, 1], DT, tag="rcol")
                nc.vector.reciprocal(rcol, ps_sc)
                nc.vector.tensor_scalar_mul(ctxn, ctxn, rcol)
                nc.sync.dma_start(out=o_ctx[:], in_=ctxn)
                nc.vector.tensor_scalar_mul(scores, scores, rcol)
                nc.sync.dma_start(out=o_scores[:], in_=scores)
                for k in range(DK):
                    ps_t = pn.tile([128, BL], DT, tag="ps_tc")
                    nc.tensor.transpose(ps_t, ctxn[:, bass.ts(k, 128)],
                                        ident[:BL, :BL])
                    nc.vector.tensor_copy(ctxT[:, k], ps_t)

            # ---------------- rnn_in + LSTMs + mel ----------------
            with tc.tile_pool(name="psum_l", bufs=1, space="PSUM") as pl, \
                 tc.tile_pool(name="psum_t2", bufs=2, space="PSUM") as pt:

                def transpose_nat(src_nat, tag, dt=DT):
                    """[16, 512] natural -> [128, LK, 16] column tiles."""
                    dst = work.tile([128, LK, BL], dt, tag=tag)
                    for k in range(LK):
                        ps_t = pt.tile([128, BL], DT, tag="ps_tr")
                        nc.tensor.transpose(ps_t, src_nat[:, bass.ts(k, 128)],
                                            ident[:BL, :BL])
                        nc.vector.tensor_copy(dst[:, k], ps_t)
                    return dst

                ps_x = pl.tile([BL, L], DT, tag="ps_x")
                xin2 = [ctxT[:, 0], ctxT[:, 1], attnhT[:, 0], attnhT[:, 1]]
                for k in range(4):
                    nc.tensor.matmul(ps_x, xin2[k], wrnin[:, k],
                                     start=(k == 0), stop=(k == 3))
                x_nat = work.tile([BL, L], DT, tag="x_nat")
                nc.vector.tensor_add(x_nat, ps_x, brnin_n)
                xT = transpose_nat(x_nat, "xT", BF)

                def lstm_cell(tag, d_wih, d_whh, xT_t, hT_t, c_nat, bl_n,
                              o_h, o_c):
                    # stream weights per k-tile: [128, 2048] (1 MB) chunks
                    ps_g = pl.tile([BL, 4 * L], DT, tag="ps_g")
                    for phase, (d_w, lhs) in enumerate(
                            [(d_wih, xT_t), (d_whh, hT_t)]):
                        wv = d_w.rearrange("(k p) n -> k p n", p=128)
                        for k in range(LK):
                            w = lstmw.tile([128, 4 * L], BF, tag="wlstm")
                            nc.gpsimd.dma_start(out=w, in_=wv[k])
                            for c in range(4):
                                nc.tensor.matmul(
                                    ps_g[:, bass.ts(c, 512)], lhs[:, k],
                                    w[:, bass.ts(c, 512)],
                                    start=(phase == 0 and k == 0),
                                    stop=(phase == 1 and k == LK - 1))
                    si = work.tile([BL, L], DT, tag="si")
                    sf = work.tile([BL, L], DT, tag="sf")
                    tg = work.tile([BL, L], DT, tag="tg")
                    so = work.tile([BL, L], DT, tag="so")
                    tb = work.tile([BL, L], DT, tag="tb")
                    nc.vector.tensor_add(tb, ps_g[:, :L], bl_n[:, :L])
                    nc.scalar.activation(si, tb, AF.Sigmoid)
                    nc.vector.tensor_add(tb, ps_g[:, L:2 * L], bl_n[:, L:2 * L])
                    nc.scalar.activation(sf, tb, AF.Sigmoid)
                    nc.vector.tensor_add(tb, ps_g[:, 2 * L:3 * L],
                                         bl_n[:, 2 * L:3 * L])
                    nc.scalar.activation(tg, tb, AF.Tanh)
                    nc.vector.tensor_add(tb, ps_g[:, 3 * L:], bl_n[:, 3 * L:])
                    nc.scalar.activation(so, tb, AF.Sigmoid)
                    c_new = work.tile([BL, L], DT, tag=f"c_new_{tag}")
                    nc.vector.tensor_mul(sf, sf, c_nat)
                    nc.vector.tensor_mul(si, si, tg)
                    nc.vector.tensor_add(c_new, sf, si)
                    nc.sync.dma_start(out=o_c[:], in_=c_new)
                    h_new = work.tile([BL, L], DT, tag=f"h_new_{tag}")
                    nc.scalar.activation(tg, c_new, AF.Tanh)
                    nc.vector.tensor_mul(h_new, so, tg)
                    nc.sync.dma_start(out=o_h[:], in_=h_new)
                    return h_new

                h1 = lstm_cell("l1", d_wl1ihT, d_wl1hhT, xT, r1hT, r1c, bl1_n,
                               o_r1h, o_r1c)
                x2 = work.tile([BL, L], DT, tag="x2")
                nc.vector.tensor_add(x2, x_nat, h1)
                x2T = transpose_nat(x2, "x2T", BF)
                h2 = lstm_cell("l2", d_wl2ihT, d_wl2hhT, x2T, r2hT, r2c, bl2_n,
                               o_r2h, o_r2c)
                x3 = work.tile([BL, L], DT, tag="x3")
                nc.vector.tensor_add(x3, x2, h2)
                x3T = transpose_nat(x3, "x3T")

                # mel projection (no bias), NMEL = 80*r columns
                ps_m = pl.tile([BL, NMEL], DT, tag="ps_m")
                for k in range(LK):
                    nc.tensor.matmul(ps_m, x3T[:, k], wmel[:, k],
                                     start=(k == 0), stop=(k == LK - 1))
                mel_nat = work.tile([BL, NMEL], DT, tag="mel_nat")
                nc.vector.tensor_copy(mel_nat, ps_m)
                nc.sync.dma_start(out=o_mels[:], in_=mel_nat)

    nc.finalize()
    return nc


_CACHE = {}
# test harness hooks: extra kwargs for run_bass_kernel_spmd (e.g. trace=True)
# and the last BassKernelResults (for exec_time_ns). The grading harness
# calls kernel() directly and never touches these.
_RUN_KWARGS = {}
_LAST_RESULT = [None]


def _get_nc(r: int):
    if r not in _CACHE:
        _CACHE[r] = _build(r)
    return _CACHE[r]


def kernel(**inputs):
    f32 = lambda x: np.ascontiguousarray(np.asarray(x), dtype=np.float32)
    r = int(np.asarray(inputs["r"]))

    enc_full = f32(inputs["encoder_seq"])
    esp_full = f32(inputs["encoder_seq_proj"])
    prenet = f32(inputs["prenet_in"])
    attnh = f32(inputs["attn_hidden"])
    r1h_, r2h_ = f32(inputs["rnn1_hidden"]), f32(inputs["rnn2_hidden"])
    r1c_, r2c_ = f32(inputs["rnn1_cell"]), f32(inputs["rnn2_cell"])
    ctxv = f32(inputs["context_vec"])

    r_build = 1 if r == 0 else r
    nc = _get_nc(r_build)

    # ---- shared (replicated) weight prep ----
    wT = lambda w: np.ascontiguousarray(f32(w).T)
    wfc1T = np.zeros((128, 256), np.float32)
    wfc1T[:80] = f32(inputs["fc1_W"]).T
    wfc2T = wT(inputs["fc2_W"])
    wattnT = wT(inputs["attn_W"])
    wgihT = wT(inputs["gru_Wih"])
    wghhT = wT(inputs["gru_Whh"])
    wrninT = wT(inputs["rnn_in_W"])
    bf = ml_dtypes.bfloat16
    wl1ihT, wl1hhT = wT(inputs["l1_Wih"]).astype(bf), wT(inputs["l1_Whh"]).astype(bf)
    wl2ihT, wl2hhT = wT(inputs["l2_Wih"]).astype(bf), wT(inputs["l2_Whh"]).astype(bf)
    mel_W = f32(inputs["mel_W"])
    idx = (np.arange(NM)[:, None] * MAXR + np.arange(r_build)[None, :]).ravel()
    wmelT = np.ascontiguousarray(mel_W[idx].T)
    vcol = np.ascontiguousarray(f32(inputs["attn_v"]).reshape(D)).astype(bf)
    bfc1 = f32(inputs["fc1_b"])
    bfc2 = f32(inputs["fc2_b"])
    bgih, bghh = f32(inputs["gru_bih"]), f32(inputs["gru_bhh"])
    brz = bgih[:2 * D] + bghh[:2 * D]
    bgin, bghn = bgih[2 * D:], bghh[2 * D:]
    brnin = f32(inputs["rnn_in_b"])
    bl1 = f32(inputs["l1_bih"]) + f32(inputs["l1_bhh"])
    bl2 = f32(inputs["l2_bih"]) + f32(inputs["l2_bhh"])

    shared = dict(
        wfc1T=wfc1T, wfc2T=wfc2T, wattnT=wattnT, wgihT=wgihT, wghhT=wghhT,
        wrninT=wrninT, wl1ihT=wl1ihT, wl1hhT=wl1hhT, wl2ihT=wl2ihT,
        wl2hhT=wl2hhT, wmelT=wmelT, vcol=vcol, bfc1=bfc1, bfc2=bfc2,
        brz=brz, bgin=bgin, bghn=bghn, brnin=brnin, bl1=bl1, bl2=bl2)

    in_maps = []
    for c in range(NCORES):
        s = slice(c * BL, (c + 1) * BL)
        pT = np.zeros((128, BL), np.float32)
        pT[:80] = prenet[s].T
        m = dict(shared)
        m["espT"] = np.ascontiguousarray(
            esp_full[s].transpose(0, 2, 1)).astype(ml_dtypes.bfloat16)
        m["enc"] = enc_full[s].astype(ml_dtypes.bfloat16)
        m["prenetT"] = pT
        m["attnhT"] = np.ascontiguousarray(attnh[s].T)
        m["attnh_nat"] = np.ascontiguousarray(attnh[s])
        m["ctxvT"] = np.ascontiguousarray(ctxv[s].T)
        m["r1hT"] = np.ascontiguousarray(r1h_[s].T).astype(ml_dtypes.bfloat16)
        m["r2hT"] = np.ascontiguousarray(r2h_[s].T).astype(ml_dtypes.bfloat16)
        m["r1c"] = np.ascontiguousarray(r1c_[s])
        m["r2c"] = np.ascontiguousarray(r2c_[s])
        in_maps.append(m)

    res = run_bass_kernel_spmd(nc, in_maps, core_ids=list(range(NCORES)),
                               **_RUN_KWARGS)
    _LAST_RESULT[0] = res
    rs = res.results

    cat = lambda k: np.concatenate([rs[c][k] for c in range(NCORES)], axis=0)
    mels = cat("o_mels").reshape(B, NM, r_build)[:, :, :r]
    scores_t = cat("o_scores")[:, None, :]
    return (mels, scores_t, cat("o_attnh"), cat("o_r1h"), cat("o_r2h"),
            cat("o_r1c"), cat("o_r2c"), cat("o_ctx"))
